# revision 40
# baseline (speedup 1.0000x reference)
"""Trainium2 Bass kernel for the GemNet AtomUpdateBlock (gnn message passing).

v2 strategy (no collectives):
  * Host: stable-sort edges by destination atom, shard the (padded) atom
    range across 8 cores.  Within each 128-atom block, each atom's edge
    list is padded to EVEN length and consecutive edge pairs share one
    "slot": a pair-tile holds 128 slots x 2 edges x 256 feats.  One fp8
    one-hot [slot -> atom] stationary then serves BOTH edges of every
    slot (half the LDWEIGHTS + half the one-hot DMA of v1).
  * Gate: rbf^T for 4 pair-tiles (8 edge sets) is packed into one
    [128,128] stationary (4 row-groups x {even 16 rows, odd 16 rows});
    a block-diagonal [[W,0],[0,W]] moving operand computes a pair-tile's
    full [128 slot, 512] gate in ONE N=512 matmul (rotating row-groups
    let LDWEIGHTS pull ahead of in-flight matmuls).
  * x = m .* gate on DVE; alternate supers route the PSUM->SBUF gate
    copy through ScalarE so the DVE multiply runs in 2x bf16 mode.
  * Scatter: one accumulating N=512 matmul per pair-tile into a
    per-block [128 atom, 512] split accumulator (even|odd halves,
    summed + cast to bf16 at block end).
  * MLP over groups of 4 blocks (512 atoms, N=512 matmuls), 7 layers,
    interleaved between edge supers via a step queue (also defers the
    PE transposes so the in-order PE stream never waits on DVE).
  * Software-pipelined: gate AND multiply produced 2 supers ahead of
    the consuming scatter; ~25 REAL matmuls on the weight
    constant pre-warm the PE HAM clock gate during the initial DMA fill
    (PE-mode transposes do not register as PE-busy for HAM).
  * Host bin-packs ATOMS into the 400 blocks (snake deal on sorted
    pair counts + move/swap overflow repair) so nearly every block
    fills its 10x128 pair-slot budget exactly: shared t_list
    [11]*4+[10]*46, T=504 computed tiles (the last DMA chunk is
    shipped padded but its pad tiles are never computed).
  * Final scale on DVE, bf16 output tensor (host casts to f32).
Precision: bf16 on TensorE paths, f32 accumulation everywhere.
"""

import sys, types, contextlib, ctypes, math
from collections import deque

sys.path.insert(0, "/opt/trn_rl_repo")

import numpy as np
import ml_dtypes

BF16_NP = ml_dtypes.bfloat16


def _install_ntff_hook_shim():
    """bass_utils imports antenv.axon_hooks for trace=True under axon; this
    container's antenv lacks that module.  Recreate the boot hook via ctypes."""
    if "antenv.axon_hooks" in sys.modules:
        return
    try:
        lib = ctypes.CDLL("/opt/axon/libaxon_pjrt.so")
    except OSError:
        lib = None
    hook = None
    if lib is not None and hasattr(lib, "axon_start_nrt_profile"):
        lib.axon_start_nrt_profile.argtypes = [ctypes.POINTER(ctypes.c_int64), ctypes.c_size_t]
        lib.axon_start_nrt_profile.restype = ctypes.c_int64
        lib.axon_stop_nrt_profile.argtypes = [ctypes.c_char_p]
        lib.axon_stop_nrt_profile.restype = ctypes.c_int64

        @contextlib.contextmanager
        def hook(output_dir, device_ids):
            import jax
            jax.devices()
            if device_ids:
                ids = (ctypes.c_int64 * len(device_ids))(*device_ids)
                rc = lib.axon_start_nrt_profile(ids, len(device_ids))
            else:
                rc = lib.axon_start_nrt_profile(None, 0)
            if rc != 0:
                raise RuntimeError(f"axon_start_nrt_profile rc={rc}")
            try:
                yield
            finally:
                n = lib.axon_stop_nrt_profile(str(output_dir).encode())
                print(f"ntff profile: {n} file(s) -> {output_dir}", file=sys.stderr)

    mod = types.ModuleType("antenv.axon_hooks")
    mod.get_axon_ntff_profile_hook = lambda: hook
    mod.set_axon_ntff_profile_hook = lambda h: None
    sys.modules["antenv.axon_hooks"] = mod


_install_ntff_hook_shim()

import concourse.bass as bass
import concourse.tile as tile
import concourse.mybir as mybir
from concourse import bacc
from concourse.alu_op_type import AluOpType
from concourse.bass_utils import run_bass_kernel_spmd

F32 = mybir.dt.float32
BF16 = mybir.dt.bfloat16
FP8 = mybir.dt.float8e4
FP8_NP = ml_dtypes.float8_e4m3fn

N_CORES = 8
N_ATOMS = 50000
N_BLOCKS = 50                  # 128-atom blocks per core
ATOMS_PER_CORE = N_BLOCKS * 128
N_ATOMS_PAD = N_CORES * ATOMS_PER_CORE
D = 256                        # feature dim
DR = 16                        # rbf dim
TILE_S = 128                   # slots per pair-tile (2 edges per slot)
SUPER = 2                      # pair-tiles per super
CHUNK = 16                     # pair-tiles per DMA chunk
N_HID = 3

# MLP groups of 4 blocks (512 atoms); 50 blocks -> 12 quads + 1 pair
GROUPS = [list(range(4 * g, 4 * g + 4)) for g in range(12)] + [[48, 49]]
NG = len(GROUPS)
GROUP_OF = {}
for _gi, _bs in enumerate(GROUPS):
    for _h, _b in enumerate(_bs):
        GROUP_OF[_b] = (_gi, _h)

INV_SQRT2 = 0.7071067811865476
S_SILU = 1.0 / 0.6

TRACE = False                  # test.py sets this for profiled runs
ACT_FUNC = "Silu"              # sim_test swaps to Sigmoid (sim lacks Silu)

_cache = {}


# ----------------------------------------------------------------- builder --
def _build(t_list):
    """Build + compile the per-core Bass graph for block pair-tile counts."""
    T = int(sum(t_list))           # computed tiles (may end mid-chunk)
    assert T % SUPER == 0
    C = (T + CHUNK - 1) // CHUNK   # shipped chunks (pad tiles are not computed)
    S = T // SUPER
    SUP_PER_CHUNK = CHUNK // SUPER
    # chunk columns (bf16): m (CHUNK pair-tiles x [2,256]) | rbf quad packs
    M_W = CHUNK * 2 * D
    RB_W = (CHUNK // 4) * TILE_S
    MD_W = M_W + RB_W
    OH_W = CHUNK * TILE_S

    block_of = np.repeat(np.arange(N_BLOCKS), t_list)
    ends = np.cumsum(t_list)
    starts = ends - np.asarray(t_list)

    nc = bacc.Bacc("TRN2", target_bir_lowering=False, debug=False,
                   num_devices=N_CORES)

    md_d = nc.dram_tensor("md", [C, 128, MD_W], BF16, kind="ExternalInput")
    oh_d = nc.dram_tensor("oh", [C, 128, OH_W], FP8, kind="ExternalInput")
    wmlp_d = nc.dram_tensor("wmlp", [128, 7 * 4 * 128], BF16, kind="ExternalInput")
    wrb_d = nc.dram_tensor("wrb", [128, 2 * D], BF16, kind="ExternalInput")
    ident_d = nc.dram_tensor("ident", [128, 128], BF16, kind="ExternalInput")
    out_d = nc.dram_tensor("out", [NG, 2, 128, 512], BF16,
                           kind="ExternalOutput")

    from contextlib import ExitStack

    with tile.TileContext(nc) as tc, ExitStack() as ctx:
        io_pool = ctx.enter_context(tc.tile_pool(name="io", bufs=4))
        ohio_pool = ctx.enter_context(tc.tile_pool(name="ohio", bufs=3))
        x_pool = ctx.enter_context(tc.tile_pool(name="x", bufs=6))
        gsb_pool = ctx.enter_context(tc.tile_pool(name="gsb", bufs=4))
        cst_pool = ctx.enter_context(tc.tile_pool(name="cst", bufs=1))
        x2sb_pool = ctx.enter_context(tc.tile_pool(name="x2sb", bufs=3))
        pairx_pool = ctx.enter_context(tc.tile_pool(name="pairx", bufs=6))
        act_pool = ctx.enter_context(tc.tile_pool(name="acts", bufs=8))
        outt_pool = ctx.enter_context(tc.tile_pool(name="outt", bufs=3))
        gate_pool = ctx.enter_context(tc.tile_pool(name="gate", bufs=2, space="PSUM"))
        x2_pool = ctx.enter_context(tc.tile_pool(name="x2", bufs=2, space="PSUM"))
        mlp_pool = ctx.enter_context(tc.tile_pool(name="mlppsum", bufs=2, space="PSUM"))

        wmlp_sb = cst_pool.tile([128, 7 * 4 * 128], BF16, tag="wmlp")
        nc.sync.dma_start(out=wmlp_sb[:], in_=wmlp_d[:])
        wrb_sb = cst_pool.tile([128, 2 * D], BF16, tag="wrb")
        nc.sync.dma_start(out=wrb_sb[:], in_=wrb_d[:])
        ident_sb = cst_pool.tile([128, 128], BF16, tag="ident")
        nc.sync.dma_start(out=ident_sb[:], in_=ident_d[:])

        # warm the PE HAM clock gate during the initial DMA fill with REAL
        # matmuls on the tiny weight constant (PE-mode transposes do NOT
        # register as PE-busy for HAM): ~3.4us of sustained MMs flips the
        # clock to 2.4 GHz before the first edge super issues.
        for _wi in range(25):
            wps = mlp_pool.tile([128, 512], F32, tag="mlppsum")
            nc.tensor.matmul(wps[:], wrb_sb[0:32, 0:128], wrb_sb[0:32, :],
                             start=True, stop=True)

        gammas = [1.0, math.sqrt(2.0), 2.0]          # s/alpha_i
        alpha4 = S_SILU * INV_SQRT2 ** 3

        def w_ap(l, kc, oc):
            i = (l * 4 + kc * 2 + oc) * 128
            return wmlp_sb[:, i:i + 128]

        silu_fn = getattr(mybir.ActivationFunctionType, ACT_FUNC)

        # ---------------- MLP group state machine ----------------
        grp_state = {}     # gi -> dict(X=[2 tiles], cur, Xres)
        steps = deque()    # pending closures, one emitted per super

        def on_block_done(b, x2_ps):
            # x2 PSUM [128 atoms, 512] f32 (even|odd halves) -> add halves ->
            # SBUF bf16 (inline, DVE).  The PE transposes are DEFERRED via the
            # step queue so the in-order PE stream never waits on the DVE add.
            gi, h = GROUP_OF[b]
            w = 128 * len(GROUPS[gi])
            if h == 0:
                xp0 = pairx_pool.tile([128, 512], BF16, tag="pairx")
                xp1 = pairx_pool.tile([128, 512], BF16, tag="pairx")
                grp_state[gi] = {"X": [xp0, xp1], "w": w}
            x2ev = x2sb_pool.tile([128, D], BF16, tag="x2ev")
            nc.scalar.copy(x2ev[:], x2_ps[:, 0:D])
            x2sb = x2sb_pool.tile([128, D], BF16, tag="x2sb")
            nc.vector.tensor_tensor(x2sb[:], x2ev[:], x2_ps[:, D:2 * D],
                                    AluOpType.add)
            steps.append(lambda gi=gi, h=h, x2sb=x2sb: emit_transp(gi, h, x2sb))
            if h == len(GROUPS[gi]) - 1:
                for l in range(7):
                    steps.append(lambda gi=gi, l=l: emit_layer(gi, l))
                steps.append(lambda gi=gi: emit_final(gi))

        def emit_transp(gi, h, x2sb):
            st = grp_state[gi]
            for cidx in range(2):
                tp = mlp_pool.tile([128, 128], BF16, tag="mlppsum")
                nc.tensor.transpose(tp[:], x2sb[:, cidx * 128:(cidx + 1) * 128],
                                    ident_sb[:])
                if cidx == 0:
                    nc.vector.tensor_copy(
                        st["X"][cidx][:, h * 128:(h + 1) * 128], tp[:])
                else:
                    nc.scalar.copy(
                        st["X"][cidx][:, h * 128:(h + 1) * 128], tp[:])

        def emit_layer(gi, l):
            st = grp_state[gi]
            w = st["w"]
            cur = st["cur"] if l > 0 else st["X"]
            new = []
            for oc in range(2):
                z = mlp_pool.tile([128, 512], F32, tag="mlppsum")
                for kc in range(2):
                    nc.tensor.matmul(z[:, :w], w_ap(l, kc, oc), cur[kc][:, :w],
                                     start=(kc == 0), stop=(kc == 1))
                h = act_pool.tile([128, 512], BF16, tag="acts")
                nc.scalar.activation(h[:, :w], z[:, :w], silu_fn)
                new.append(h)
            if l == 0:
                st["Xres"] = new
                st["cur"] = new
            elif l % 2 == 1:               # A-layer output
                st["cur"] = new
            else:                          # B-layer output: residual
                i_res = l // 2 - 1
                nxt = []
                for cidx in range(2):
                    xn = act_pool.tile([128, 512], BF16, tag="acts")
                    nc.vector.scalar_tensor_tensor(
                        xn[:, :w], new[cidx][:, :w], gammas[i_res],
                        st["Xres"][cidx][:, :w],
                        AluOpType.mult, AluOpType.add)
                    nxt.append(xn)
                st["Xres"] = nxt
                st["cur"] = nxt

        def emit_final(gi):
            st = grp_state.pop(gi)
            w = st["w"]
            for cidx in range(2):
                ot = outt_pool.tile([128, 512], BF16, tag="outt")
                nc.vector.tensor_scalar_mul(ot[:, :w], st["Xres"][cidx][:, :w],
                                            alpha4)
                nc.gpsimd.dma_start(out=out_d[gi, cidx, :, 0:w], in_=ot[:, :w])

        # ---------------- edge phase (software pipelined) ----------------
        md_tiles = {}
        oh_tiles = {}

        def emit_dma(c):
            if c in md_tiles or c >= C:
                return
            t_io = io_pool.tile([128, MD_W], BF16, tag="io")
            nc.sync.dma_start(out=t_io[:], in_=md_d[c])
            md_tiles[c] = t_io
            t_oh = ohio_pool.tile([128, OH_W], FP8, tag="ohio")
            nc.scalar.dma_start(out=t_oh[:], in_=oh_d[c])
            oh_tiles[c] = t_oh

        gate_tiles = {}

        def emit_gate(s):
            c = s // SUP_PER_CHUNK
            if s % SUP_PER_CHUNK == 0:
                emit_dma(c)
                emit_dma(c + 1)
            md = md_tiles[c]
            gate = gate_pool.tile([128, SUPER * 2 * D], F32, tag="gate")
            for jj in range(SUPER):
                t = s * SUPER + jj
                g = t % 4
                q = (t % CHUNK) // 4
                rbf_stat = md[32 * g:32 * g + 32,
                              M_W + q * TILE_S:M_W + (q + 1) * TILE_S]
                nc.tensor.matmul(gate[:, jj * 512:(jj + 1) * 512], rbf_stat,
                                 wrb_sb[32 * g:32 * g + 32, :],
                                 tile_position=(32 * g, 0))
            gate_tiles[s] = gate

        xsb_tiles = {}

        def emit_mult(s):
            c = s // SUP_PER_CHUNK
            md = md_tiles[c]
            sp = s % SUP_PER_CHUNK
            gate = gate_tiles.pop(s)
            xsb = x_pool.tile([128, SUPER * 2 * D], BF16, tag="x")
            m_v = md[:, sp * SUPER * 2 * D:(sp + 1) * SUPER * 2 * D]
            if s % 2 == 0:
                # offload the PSUM read to ScalarE: gate -> SBUF bf16, then
                # the VectorE multiply runs in 2x mode (all-bf16 SBUF)
                gsb = gsb_pool.tile([128, SUPER * 2 * D], BF16, tag="gsb")
                nc.scalar.copy(gsb[:], gate[:])
                nc.vector.tensor_tensor(xsb[:], m_v, gsb[:], AluOpType.mult)
            else:
                # with the multiply 2 supers ahead of its scatter there is no
                # need to split halves for an early scatter start: one
                # FD=1024 op saves the second instruction's fixed cost
                nc.vector.tensor_tensor(xsb[:], m_v, gate[:], AluOpType.mult)
            xsb_tiles[s] = xsb

        x2_cur = None
        emit_gate(0)
        emit_mult(0)
        emit_gate(1)
        emit_mult(1)
        for s in range(S):
            if s + 2 < S:
                emit_gate(s + 2)
                emit_mult(s + 2)
            c = s // SUP_PER_CHUNK
            xsb = xsb_tiles.pop(s)
            for jj in range(SUPER):
                t = s * SUPER + jj
                b = int(block_of[t])
                oh = oh_tiles[c][:, TILE_S * (t % CHUNK):TILE_S * (t % CHUNK + 1)]
                if t == starts[b]:
                    x2_cur = x2_pool.tile([128, 2 * D], F32, tag="x2")
                last = (t == ends[b] - 1)
                nc.tensor.matmul(x2_cur[:], oh,
                                 xsb[:, jj * 512:(jj + 1) * 512],
                                 start=(t == starts[b]), stop=last)
                if last:
                    on_block_done(b, x2_cur)
            if steps:
                steps.popleft()()
            if steps and s >= S - 40:
                steps.popleft()()
        # keep-warm during the serial MLP drain: one tiny dep-free MM per
        # step fills the inter-layer sem-wait so the HAM MID window never
        # sees the PE as idle (N=128 ~= 220ns, hidden in the ~1us waits).
        warm_tail = x2_pool.tile([128, 2 * D], F32, tag="x2")
        while steps:
            steps.popleft()()
            nc.tensor.matmul(warm_tail[:, 0:128], wrb_sb[0:32, 0:128],
                             wrb_sb[0:32, 0:128], start=True, stop=True)

    nc.compile()
    return nc


# ------------------------------------------------------------ host wrapper --
def kernel(h=None, m=None, rbf=None, id_j=None, W_rbf=None, W1=None,
           res_W=None, scale=None, **_unused):
    global LAST_RESULT
    m = np.ascontiguousarray(np.asarray(m, dtype=np.float32))
    rbf = np.ascontiguousarray(np.asarray(rbf, dtype=np.float32))
    ids = np.asarray(id_j).astype(np.int64)
    W_rbf = np.asarray(W_rbf, dtype=np.float32)
    W1 = np.asarray(W1, dtype=np.float32)
    res_W = np.asarray(res_W, dtype=np.float32)
    scale_v = float(np.asarray(scale).reshape(-1)[0])

    nE = ids.shape[0]

    # ---- sort edges by destination atom, shard atoms across cores ----
    perm = np.argsort(ids, kind="stable")
    ids_s = ids[perm]

    # ---- bin-pack atoms into 128-atom blocks with near-exact pair fills ----
    # t_list is fixed at [11]*8 + [10]*42 (+chunk pad) for every core; atoms
    # are packed so each block's pair count fits its cap, which removes the
    # per-block ceil padding of the id-order layout (T 544 -> 512).
    NBG = N_CORES * N_BLOCKS
    cnt_atom = np.bincount(ids, minlength=N_ATOMS)
    s_cnt = (cnt_atom + 1) >> 1
    assert int(s_cnt.max()) <= 1280, "single atom exceeds block capacity"
    per_blk = N_ATOMS // NBG
    order = np.argsort(-s_cnt, kind="stable")
    deal = order[:NBG * per_blk].reshape(per_blk, NBG).copy()
    deal[1::2] = deal[1::2, ::-1]              # snake -> near-equal fills
    blk_atoms = [list(deal[:, b]) for b in range(NBG)]
    for a in order[NBG * per_blk:]:            # leftovers, if any
        bb = min(range(NBG), key=lambda b: len(blk_atoms[b]))
        blk_atoms[bb].append(a)
    fills = np.array([int(s_cnt[np.array(b)].sum()) for b in blk_atoms])
    caps = np.full(NBG, 10 * 128, np.int64)
    eleven = np.argsort(-fills, kind="stable")[:4 * N_CORES]
    caps[eleven] = 11 * 128
    for b in range(NBG):                       # shed overflow to slack blocks
        if fills[b] <= caps[b]:
            continue
        blk_atoms[b].sort(key=lambda a: s_cnt[a])
        while fills[b] > caps[b]:
            # try a plain move: donor atom of size <= receiver slack
            rb = max((x for x in range(NBG) if x != b
                      and len(blk_atoms[x]) < 128),
                     key=lambda x: caps[x] - fills[x], default=None)
            slack = caps[rb] - fills[rb] if rb is not None else 0
            cand = [a for a in blk_atoms[b] if s_cnt[a] <= slack]
            if cand:
                a = max(cand, key=lambda a: s_cnt[a])
                sz = int(s_cnt[a])
                blk_atoms[b].remove(a)
                fills[b] -= sz
                blk_atoms[rb].append(a)
                fills[rb] += sz
                continue
            # else swap donor's largest atom with a smaller one elsewhere
            a_d = max(blk_atoms[b], key=lambda a: s_cnt[a])
            s_d = int(s_cnt[a_d])
            done = False
            for x in sorted(range(NBG), key=lambda x: fills[x] - caps[x]):
                if x == b or fills[x] >= caps[x]:
                    continue
                sl = caps[x] - fills[x]
                small = [a for a in blk_atoms[x]
                         if s_cnt[a] < s_d and s_d - s_cnt[a] <= sl]
                if small:
                    a_r = min(small, key=lambda a: s_cnt[a])
                    s_r = int(s_cnt[a_r])
                    blk_atoms[b].remove(a_d)
                    blk_atoms[x].remove(a_r)
                    blk_atoms[b].append(a_r)
                    blk_atoms[x].append(a_d)
                    fills[b] -= s_d - s_r
                    fills[x] += s_d - s_r
                    done = True
                    break
            assert done, "bin-pack repair failed"

    # deal blocks to cores: elevens then tens, snake on fill
    is11 = np.zeros(NBG, bool)
    is11[eleven] = True
    core_blocks = [[] for _ in range(N_CORES)]
    for group in (eleven[np.argsort(-fills[eleven], kind="stable")],
                  np.array([b for b in np.argsort(-fills, kind="stable")
                            if not is11[b]])):
        for i, b in enumerate(group):
            r, j = divmod(i, N_CORES)
            k = j if r % 2 == 0 else N_CORES - 1 - j
            core_blocks[k].append(int(b))

    atom_core = np.empty(N_ATOMS, np.int64)
    atom_vid = np.empty(N_ATOMS, np.int64)
    for k in range(N_CORES):
        for j, b in enumerate(core_blocks[k]):
            ats = np.asarray(blk_atoms[b], dtype=np.int64)
            atom_core[ats] = k
            atom_vid[ats] = j * 128 + np.arange(len(ats))

    core_e = atom_core[ids_s]
    vid_e = atom_vid[ids_s]

    # per-core pair-slot assignment (on virtual local atom ids)
    per_core = []
    cnts_pairs = np.zeros((N_CORES, N_BLOCKS), np.int64)
    for k in range(N_CORES):
        idxk = np.nonzero(core_e == k)[0]
        idxk = idxk[np.argsort(vid_e[idxk], kind="stable")]
        ids_k = vid_e[idxk]                          # sorted virtual ids
        sel_k = perm[idxk]                           # original edge rows
        nk = len(ids_k)
        # within-atom rank
        first = np.searchsorted(ids_k, ids_k)        # index of first occ
        r = np.arange(nk) - first
        p_atom = r >> 1
        half = (r & 1).astype(np.int64)
        # slots per atom / per block
        cnt = np.bincount(ids_k, minlength=ATOMS_PER_CORE)
        s_cnt = (cnt + 1) >> 1
        off_atom = np.cumsum(s_cnt) - s_cnt          # global slot offset
        blk = ids_k >> 7
        blk_first_atom = (blk << 7)
        blk_slot_start_per_atom = off_atom[blk_first_atom]
        slot_in_block = off_atom[ids_k] - blk_slot_start_per_atom + p_atom
        sc = np.add.reduceat(s_cnt, np.arange(0, ATOMS_PER_CORE, 128))
        cnts_pairs[k] = sc
        per_core.append((ids_k, blk, slot_in_block, half, sel_k))

    t_list = np.array([11] * 4 + [10] * (N_BLOCKS - 4), np.int64)
    assert (cnts_pairs <= t_list[None, :] * TILE_S).all(), "bin-pack overflow"
    T = int(t_list.sum())                 # 504; the last DMA chunk is partial
    TP = ((T + CHUNK - 1) // CHUNK) * CHUNK   # array/tile layout size (512)
    C = TP // CHUNK
    M_W = CHUNK * 2 * D
    RB_W = (CHUNK // 4) * TILE_S
    MD_W = M_W + RB_W
    OH_W = CHUNK * TILE_S

    key = tuple(t_list.tolist())
    if key not in _cache:
        _cache[key] = _build(t_list)
    nc = _cache[key]

    offs = np.concatenate([[0], np.cumsum(t_list)[:-1]])   # tile offset per block

    # ---- shared (replicated) small tensors ----
    s_, c_ = S_SILU, INV_SQRT2
    alphas = [s_, c_ * s_, c_ * c_ * s_]
    layersW = [scale_v * W1]
    for i in range(N_HID):
        layersW.append(alphas[i] * res_W[i, 0])
        layersW.append(s_ * res_W[i, 1])
    wmlp = np.zeros((128, 7 * 4 * 128), np.float32)
    for l in range(7):
        Wl = layersW[l]
        for kc in range(2):
            for oc in range(2):
                i = (l * 4 + kc * 2 + oc) * 128
                wmlp[:, i:i + 128] = Wl[kc * 128:(kc + 1) * 128,
                                        oc * 128:(oc + 1) * 128]
    wmlp = wmlp.astype(BF16_NP)
    # block-diagonal [[W,0],[0,W]] moving operand, replicated per row-group
    wrb = np.zeros((128, 2 * D), np.float32)
    for g in range(4):
        wrb[32 * g:32 * g + DR, 0:D] = W_rbf
        wrb[32 * g + 16:32 * g + 16 + DR, D:2 * D] = W_rbf
    wrb = wrb.astype(BF16_NP)
    ident = np.eye(128, dtype=BF16_NP)

    # ---- per-core big tensors ----
    m_bf = None
    in_maps = []
    for k in range(N_CORES):
        ids_k, blk, slot_in_block, half, sel = per_core[k]
        tile_i = offs[blk] + (slot_in_block >> 7)
        part_i = slot_in_block & 127

        m_arr = np.zeros((TP, 128, 2, D), BF16_NP)
        m_arr[tile_i, part_i, half] = m[sel].astype(BF16_NP)
        rbfT = np.zeros((TP // 4, 4, 2, 16, TILE_S), BF16_NP)
        rbfT[tile_i >> 2, tile_i & 3, half, :, part_i] = rbf[sel].astype(BF16_NP)
        onehot = np.zeros((TP, 128, TILE_S), FP8_NP)
        e0 = half == 0
        onehot[tile_i[e0], part_i[e0], ids_k[e0] & 127] = 1.0

        md = np.empty((C, 128, MD_W), BF16_NP)
        md[:, :, :M_W] = m_arr.reshape(C, CHUNK, 128, 2 * D) \
            .transpose(0, 2, 1, 3).reshape(C, 128, M_W)
        md[:, :, M_W:] = rbfT.reshape(C, CHUNK // 4, 128, TILE_S) \
            .transpose(0, 2, 1, 3).reshape(C, 128, RB_W)
        oh = np.ascontiguousarray(
            onehot.reshape(C, CHUNK, 128, TILE_S)
            .transpose(0, 2, 1, 3).reshape(C, 128, OH_W))

        in_maps.append({
            "md": md, "oh": oh, "wmlp": wmlp, "wrb": wrb, "ident": ident,
        })

    res = run_bass_kernel_spmd(nc, in_maps, list(range(N_CORES)), trace=TRACE)
    LAST_RESULT = res

    # ---- reassemble (scatter block rows back to global atom ids) ----
    out = np.empty((N_ATOMS, D), np.float32)
    for k in range(N_CORES):
        od = np.asarray(res.results[k]["out"], dtype=np.float32)
        for gi, bs in enumerate(GROUPS):
            w = 128 * len(bs)
            arr = od[gi, :, :, :w]                     # [2, 128, w]
            arr = arr.reshape(2, 128, len(bs), 128).transpose(2, 3, 0, 1) \
                .reshape(len(bs) * 128, D)
            for hh, j in enumerate(bs):
                ats = np.asarray(blk_atoms[core_blocks[k][j]], dtype=np.int64)
                out[ats] = arr[hh * 128:hh * 128 + len(ats)]
    return out


LAST_RESULT = None

